# revision 7
# baseline (speedup 1.0000x reference)
"""Trainium2 Bass kernel v2 for nn_ReasoningLayer (per-token MLP, passthrough pos 0).

  out[:, 0]  = hidden_states[:, 0]
  out[:, i]  = GELU(hidden_states[:, i] @ W1 + b1) @ W2 + b2   for i >= 1

Sharding: data parallel over batch — core b computes batch b (2048 tokens).
Transposed layout on device (x^T [D, T]) so both matmuls contract over SBUF
partitions with no on-device transposes.

v2 changes vs baseline:
  - weights DMA on the ACT/Pool queues, x/y on the SP queue -> a cold
    invocation's first matmul starts after ~1MB (x slab 0 || w1 chunk 0)
    instead of after 9MB of serialized FIFO traffic
  - weight chunks ordered by first use (w1 jh0..3, then w2 halves)
  - software pipeline: mm1(slab i) issued before mm2(slab i-1) so the PE
    never waits on the tail GELU of its own slab
  - y output in bf16 (halves output DMA; adds ~0.2% relative rounding)
  - coarse DMAs (1 per x slab, 4 per weight matrix) to cut issue overhead
"""

import numpy as np
import ml_dtypes

B, S, D, H = 8, 2048, 1024, 2048
P = 128
NCORES = 8
TCORE = (B * S) // NCORES  # 2048 tokens per core
TSLAB = 512
NSLAB = TCORE // TSLAB     # 4
DO = D // P                # 8
JO = H // P                # 16
OO = D // P                # 8

COMPUTE = "bf16"

_nc_cache = {}


def _build(compute=COMPUTE, repeat=1, cold=False):
    import concourse.bass as bass
    import concourse.mybir as mybir
    import concourse.tile as tile
    from concourse import bacc

    f32 = mybir.dt.float32
    bf16 = mybir.dt.bfloat16
    cdt = bf16
    ts = bass.ts
    Gelu = mybir.ActivationFunctionType.Gelu

    nc = bacc.Bacc("TRN2", target_bir_lowering=False, debug=False,
                   num_devices=NCORES)
    xT = nc.dram_tensor("xT", [D, TCORE], cdt, kind="ExternalInput")
    w1 = nc.dram_tensor("w1", [D, H], cdt, kind="ExternalInput")
    b1 = nc.dram_tensor("b1", [H], f32, kind="ExternalInput")
    w2 = nc.dram_tensor("w2", [H, D], cdt, kind="ExternalInput")
    b2 = nc.dram_tensor("b2", [D], f32, kind="ExternalInput")
    yT = nc.dram_tensor("yT", [D, TCORE], bf16, kind="ExternalOutput")

    with tile.TileContext(nc) as tc:
        with (
            tc.tile_pool(name="w", bufs=1) as wpool,
            tc.tile_pool(name="bias", bufs=1) as bpool,
            tc.tile_pool(name="x", bufs=NSLAB) as xpool,
            tc.tile_pool(name="h", bufs=2) as hpool,
            tc.tile_pool(name="y", bufs=2) as ypool,
            tc.tile_pool(name="ps1", bufs=4, space=bass.MemorySpace.PSUM) as pp1,
            tc.tile_pool(name="ps2", bufs=4, space=bass.MemorySpace.PSUM) as pp2,
        ):
            w1_sb = wpool.tile([P, DO, H], cdt, name="w1_sb")
            w2_sb = wpool.tile([P, JO, D], cdt, name="w2_sb")
            b1_sb = bpool.tile([P, JO], f32, name="b1_sb")
            b2_sb = bpool.tile([P, OO], f32, name="b2_sb")
            # HAM warmup: dummy matmuls on a memset tile keep the PE busy
            # during the initial DMA fill so real matmuls start at full clock
            warm_sb = bpool.tile([P, 16], cdt, name="warm_sb")
            nc.vector.memset(warm_sb[:], 0.0)
            w1r = w1.rearrange("(do di) j -> di do j", di=P)
            w2r = w2.rearrange("(jo ji) o -> ji jo o", ji=P)
            xTr = xT.rearrange("(do di) t -> di do t", di=P)
            yTr = yT.rearrange("(oo oi) t -> oi oo t", oi=P)

            for rep in range(repeat):
                first = rep == 0 or cold
                if first:
                    # SP queue: biases + x slabs (first matmul needs x0 only)
                    nc.sync.dma_start(b1_sb[:], b1.rearrange("(jo ji) -> ji jo", ji=P))
                    nc.sync.dma_start(b2_sb[:], b2.rearrange("(oo oi) -> oi oo", oi=P))
                x_tiles = []
                for it in range(NSLAB):
                    x_sb = xpool.tile([P, DO, TSLAB], cdt, tag="x_sb")
                    if it == 0 and first:
                        # split the first slab's load: the first 4 k-tiles
                        # land ~2.5us earlier, so mm1 group 0 starts sooner
                        # on a cold invocation (deps are per-region)
                        nc.sync.dma_start(x_sb[:, 0:4], xTr[:, 0:4, ts(it, TSLAB)])
                        nc.sync.dma_start(x_sb[:, 4:8], xTr[:, 4:8, ts(it, TSLAB)])
                    else:
                        nc.sync.dma_start(x_sb[:], xTr[:, :, ts(it, TSLAB)])
                    x_tiles.append(x_sb)
                if first:
                    # GPSIMD (SWDGE) queue: weights, ordered by first use —
                    # runs in parallel with the SP queue's x loads and never
                    # blocks the ACT queue's activations
                    for jh in range(8):
                        nc.gpsimd.dma_start(w1_sb[:, :, ts(jh, H // 8)],
                                            w1r[:, :, ts(jh, H // 8)])
                    for oh in range(4):
                        nc.gpsimd.dma_start(w2_sb[:, :, ts(oh, D // 4)],
                                            w2r[:, :, ts(oh, D // 4)])
                    # warm the PE clock gate while the fill DMAs land
                    wps = pp1.tile([P, TSLAB], f32, tag="ps1")
                    for _ in range(40):
                        nc.tensor.matmul(wps[0:16, 0:16], warm_sb[:],
                                         warm_sb[:], start=True, stop=True)

                h_tiles = [None] * NSLAB

                def mm1(it):
                    x_sb = x_tiles[it]
                    h_sb = hpool.tile([P, JO, TSLAB], cdt, tag="h_sb")
                    for jt in range(JO):
                        ps = pp1.tile([P, TSLAB], f32, tag="ps1")
                        for kt in range(DO):
                            nc.tensor.matmul(
                                ps[:],
                                w1_sb[:, kt, ts(jt, P)],
                                x_sb[:, kt],
                                start=(kt == 0),
                                stop=(kt == DO - 1),
                            )
                        nc.scalar.activation(h_sb[:, jt], ps[:], Gelu,
                                             bias=b1_sb[:, ts(jt, 1)])
                    h_tiles[it] = h_sb

                def mm2(it):
                    h_sb = h_tiles[it]
                    y_sb = ypool.tile([P, OO, TSLAB], bf16, tag="y_sb")
                    for ot in range(OO):
                        ps2 = pp2.tile([P, TSLAB], f32, tag="ps2")
                        for jt in range(JO):
                            nc.tensor.matmul(
                                ps2[:],
                                w2_sb[:, jt, ts(ot, P)],
                                h_sb[:, jt],
                                start=(jt == 0),
                                stop=(jt == JO - 1),
                            )
                        nc.vector.tensor_scalar_add(y_sb[:, ot], ps2[:],
                                                    b2_sb[:, ts(ot, 1)])
                        # y DMA on the ACT ring: overlaps while keeping the
                        # drain tail short, and keeps the SP ring free so the
                        # next rep's x loads are never queued behind y writes.
                        # Progressively finer slices on the last slab: the
                        # final writes wait on one add each, not two.
                        if it == NSLAB - 1:
                            emit = ot in (1, 3, 5) or ot >= 6
                            lo = ot if ot >= 6 else ot - 1
                        else:
                            emit = ot % 4 == 3
                            lo = ot - 3
                        if emit:
                            nc.scalar.dma_start(
                                yTr[:, lo:ot + 1, ts(it, TSLAB)],
                                y_sb[:, lo:ot + 1],
                            )

                mm1(0)
                for it in range(1, NSLAB):
                    mm1(it)
                    mm2(it - 1)
                mm2(NSLAB - 1)

    nc.compile()
    return nc


def _get_nc(compute=COMPUTE, repeat=1, cold=False):
    key = (compute, repeat, cold)
    if key not in _nc_cache:
        _nc_cache[key] = _build(compute, repeat, cold)
    return _nc_cache[key]


def _run(hidden_states, W1, b1, W2, b2, compute=COMPUTE, trace=False):
    from concourse import bass_utils

    nc = _get_nc(compute)
    hidden_states = np.asarray(hidden_states, np.float32)
    cnp = ml_dtypes.bfloat16
    W1c = np.ascontiguousarray(np.asarray(W1).astype(cnp))
    W2c = np.ascontiguousarray(np.asarray(W2).astype(cnp))
    b1c = np.ascontiguousarray(np.asarray(b1, np.float32))
    b2c = np.ascontiguousarray(np.asarray(b2, np.float32))

    in_maps = []
    for c in range(NCORES):
        xT_c = hidden_states[c].T.astype(cnp, order="C")  # [D, TCORE]
        in_maps.append({"xT": xT_c, "w1": W1c, "b1": b1c, "w2": W2c, "b2": b2c})

    res = bass_utils.run_bass_kernel_spmd(
        nc, in_maps, core_ids=list(range(NCORES)), trace=trace
    )

    out = np.empty((B, S, D), np.float32)
    for c in range(NCORES):
        out[c] = res.results[c]["yT"].T.astype(np.float32)
    out[:, 0, :] = hidden_states[:, 0, :]
    return out, res


def kernel(hidden_states, W1, b1, W2, b2):
    out, _ = _run(hidden_states, W1, b1, W2, b2)
    return out
